# revision 1
# baseline (speedup 1.0000x reference)
"""Trainium2 Bass kernel for per-channel batched Linear:
    out[b,c,p,e] = sum_q W[e,p,q] * x[b,c,q,e] + bias[e,p]

Shapes: x [16,128,512,64] f32, W [64,512,512] f32, b [64,512] f32.

Strategy: shard embed_dim E=64 across 8 cores (8 channels/core). Each
channel is an independent GEMM out_e^T = W_e @ X_e^T with X_e = x[..,..,:,e]
flattened to [2048, 512]. We compute the transposed output [P, M] so the
weight is the matmul's stationary operand (reused across the M dim) and the
bias becomes a per-partition scalar for the PSUM->SBUF eviction op.

Host side: slice + cast fp32 -> bf16 + lay out operands so every DMA is a
big contiguous 128-partition transfer. Device: bf16 matmuls (full PE rate),
fp32 PSUM accumulate, bias added during PSUM eviction on ACT/DVE.
"""

import numpy as np
import ml_dtypes
from contextlib import ExitStack

import concourse.bass as bass
import concourse.tile as tile
from concourse import bacc, mybir
from concourse import bass_utils
from concourse.bass import ts

B, C, Q, E = 16, 128, 512, 64
P = 512            # output projection size (== Q here)
N_CORES = 8
E_LOC = E // N_CORES   # 8 channels per core
M = B * C              # 2048 rows per channel GEMM

QT = Q // 128          # 4 k-tiles
PT = P // 128          # 4 output-partition tiles
MC = M // 512          # 4 moving-dim chunks

BF16 = mybir.dt.bfloat16
F32 = mybir.dt.float32

_CACHE = {}


def _kernel_body(tc, out, xt, wt, bias_d):
    nc = tc.nc
    with ExitStack() as ctx:
        xpool = ctx.enter_context(tc.tile_pool(name="x", bufs=2))
        wpool = ctx.enter_context(tc.tile_pool(name="w", bufs=2))
        opool = ctx.enter_context(tc.tile_pool(name="o", bufs=3))
        bpool = ctx.enter_context(tc.tile_pool(name="bias", bufs=1))
        psum = ctx.enter_context(tc.tile_pool(name="psum", bufs=2, space="PSUM"))

        bias_sb = bpool.tile([128, E_LOC * PT], F32)
        nc.sync.dma_start(bias_sb[:], bias_d[:])

        for e in range(E_LOC):
            # whole channel of X^T: [128 q, QT, M] and W^T: [128 q, QT, P]
            x_sb = xpool.tile([128, QT, M], BF16, tag="x")
            nc.sync.dma_start(
                x_sb[:], xt[e].rearrange("(qt qp) m -> qp qt m", qp=128)
            )
            w_sb = wpool.tile([128, QT, P], BF16, tag="w")
            nc.sync.dma_start(
                w_sb[:], wt[e].rearrange("(qt qp) p -> qp qt p", qp=128)
            )

            for pt in range(PT):
                ps = [
                    psum.tile([128, 512], F32, name=f"ps_{mc}") for mc in range(MC)
                ]
                for qt in range(QT):
                    lhsT = w_sb[:, qt, ts(pt, 128)]
                    for mc in range(MC):
                        nc.tensor.matmul(
                            ps[mc][:],
                            lhsT,
                            x_sb[:, qt, ts(mc, 512)],
                            start=(qt == 0),
                            stop=(qt == QT - 1),
                        )
                o_sb = opool.tile([128, M], BF16, tag="o")
                bj = e * PT + pt
                bias_ap = bias_sb[:, bj : bj + 1]
                for mc in range(MC):
                    # alternate eviction between ACT and DVE so neither is
                    # the bottleneck
                    if mc % 2 == 0:
                        nc.scalar.activation(
                            o_sb[:, ts(mc, 512)],
                            ps[mc][:],
                            mybir.ActivationFunctionType.Identity,
                            bias=bias_ap,
                        )
                    else:
                        nc.vector.tensor_scalar_add(
                            o_sb[:, ts(mc, 512)], ps[mc][:], bias_ap
                        )
                nc.sync.dma_start(out[e, ts(pt, 128)], o_sb[:])


def _build():
    if "nc" in _CACHE:
        return _CACHE["nc"]
    nc = bacc.Bacc(
        "TRN2",
        target_bir_lowering=False,
        debug=False,
        enable_asserts=True,
        num_devices=N_CORES,
    )
    xt = nc.dram_tensor("xt", [E_LOC, Q, M], BF16, kind="ExternalInput").ap()
    wt = nc.dram_tensor("wt", [E_LOC, Q, P], BF16, kind="ExternalInput").ap()
    bias_d = nc.dram_tensor("bias", [128, E_LOC * PT], F32, kind="ExternalInput").ap()
    out = nc.dram_tensor("out", [E_LOC, P, M], BF16, kind="ExternalOutput").ap()
    with tile.TileContext(nc) as tc:
        _kernel_body(tc, out, xt, wt, bias_d)
    nc.compile()
    _CACHE["nc"] = nc
    return nc


def make_in_maps(x, W, b):
    """Host-side shard + cast + layout. Returns list of 8 per-core dicts."""
    in_maps = []
    for r in range(N_CORES):
        e0 = r * E_LOC
        e1 = e0 + E_LOC
        # X^T per channel: [e, q, m] where m = b*C + c
        xs = (
            x[:, :, :, e0:e1]
            .transpose(3, 2, 0, 1)
            .astype(ml_dtypes.bfloat16)
            .reshape(E_LOC, Q, M)
        )
        # W^T per channel: [e, q, p]
        ws = np.ascontiguousarray(W[e0:e1].transpose(0, 2, 1)).astype(
            ml_dtypes.bfloat16
        )
        # bias laid out [128, e*PT + pt] with partition = p % 128
        bs = np.ascontiguousarray(
            b[e0:e1].reshape(E_LOC, PT, 128).transpose(2, 0, 1)
        ).reshape(128, E_LOC * PT)
        in_maps.append({"xt": xs, "wt": ws, "bias": np.ascontiguousarray(bs, np.float32)})
    return in_maps


def assemble_output(results):
    out = np.empty((B, C, P, E), np.float32)
    for r in range(N_CORES):
        o = results[r]["out"]  # [E_LOC, P, M] bf16
        out[:, :, :, r * E_LOC : (r + 1) * E_LOC] = (
            o.astype(np.float32).transpose(2, 1, 0).reshape(B, C, P, E_LOC)
        )
    return out


def run_on_hw(x, W, b, **spmd_kwargs):
    nc = _build()
    in_maps = make_in_maps(x, W, b)
    res = bass_utils.run_bass_kernel_spmd(
        nc, in_maps, core_ids=list(range(N_CORES)), **spmd_kwargs
    )
    return assemble_output(res.results), res


def kernel(x, W, b):
    out, _ = run_on_hw(x, W, b)
    return out
